# revision 48
# baseline (speedup 1.0000x reference)
"""Trainium2 Bass kernel for nn_Colar_static (retrieval_knn).

Strategy: data-parallel over batch B across 8 cores PLUS tensor-parallel
split of the Ek/Ev prototype projections over the C=1024 channel dim
(each core computes a [128, K*N] slab = 1/8 of the work the baseline
replicated). Slabs are exchanged with on-chip collectives:
  - AllReduce  [2, KN] f32   : Ek column sum-of-squares + wEv gate row
  - AllGather  [128, KN] bf16: Ek slab (c-tile per rank)
  - AllGather  [128, KN] bf16: Ev^T slab (kn on partitions, c-slice free)
Collectives overlap the batch-local k/v projection phase.

DMA descriptor *triggers* cost ~607ns each, serialized on the issuing
engine (SP or Activation are the only HW-DGE triggers). So all bulk
traffic uses host-retiled [128, i, n] layouts so each logical load is
ONE multi-dim DMA, and triggers are spread across the sync and scalar
queues. KV accumulates each PSUM bank to completion (q-outer) to avoid
the bank-cycling micro-idle penalty.

SBUF singles are created in reverse order of death (LIFO pool stack).
"""

import sys

for _p in ("/opt/trn_rl_repo", "/opt/pypackages"):
    if _p not in sys.path:
        sys.path.append(_p)

import numpy as np
import ml_dtypes

import concourse.bass as bass
import concourse.mybir as mybir
import concourse.tile as tile
from concourse import bacc
from concourse import bass_utils

B, T, CH, C, N, K = 4096, 8, 2048, 1024, 512, 5
NCORES = 8
BL = B // NCORES            # 512 batch rows per core
KN = K * N                  # 2560 prototype columns
P = 128
NT_I = CH // P              # 16 contraction tiles (input channels)
NT_C = C // P               # 8 tiles over C
NT_KN = KN // P             # 20 tiles over K*N
NT_KV = 2 * C // P          # 16 tiles over [k|v] output channels
TPK = NT_KN // K            # 4 kn-tiles per prototype
NCH = KN // 512             # 5 column chunks for the slab projections
EPS = 1e-8

F32 = mybir.dt.float32
BF16 = mybir.dt.bfloat16
AF = mybir.ActivationFunctionType

_CACHE = {}


def _build_nc():
    nc = bacc.Bacc(None, target_bir_lowering=False, debug=False)

    # [P, NT_I, n] host-retiled layouts: row (i*P + p) -> [p, i]
    xTt = nc.dram_tensor("xTt", [P, NT_I, BL], BF16, kind="ExternalInput")
    wkvt = nc.dram_tensor("wkvt", [P, NT_I, 2 * C], BF16,
                          kind="ExternalInput")
    wekt = nc.dram_tensor("wekt", [P, NT_I, P], BF16, kind="ExternalInput")
    wevt = nc.dram_tensor("wevt", [P, NT_I, P], BF16, kind="ExternalInput")
    statt = nc.dram_tensor("statt", [P, NT_I, KN], BF16,
                           kind="ExternalInput")
    bekc = nc.dram_tensor("bekc", [P, 1], F32, kind="ExternalInput")
    bevc = nc.dram_tensor("bevc", [P, 1], F32, kind="ExternalInput")
    wwc = nc.dram_tensor("wwc", [P, 1], BF16, kind="ExternalInput")
    bkv = nc.dram_tensor("bkv", [P, NT_KV], F32, kind="ExternalInput")
    ident = nc.dram_tensor("ident", [P, P], BF16, kind="ExternalInput")
    wout = nc.dram_tensor("wout", [P, NT_KV * K], BF16, kind="ExternalInput")
    bws = nc.dram_tensor("bws", [1, 1], F32, kind="ExternalInput")
    boutt = nc.dram_tensor("boutt", [K, 1], F32, kind="ExternalInput")
    outT = nc.dram_tensor("outT", [K, BL], F32, kind="ExternalOutput")

    # one packed collective buffer per rank:
    #   rows 0..127   Ek slab, row 128 sum-of-squares partial (bf16),
    #   row 129       wEv gate partial, rows 130..257 Ev^T slab.
    # ONE AllGather = one rank barrier + one ring phase; output Shared
    # so ranks deposit slices into a single HBM buffer.
    G1 = P + 2
    ccg_in = nc.dram_tensor("ccg_in", [G1, KN], BF16)
    ccg_out = nc.dram_tensor("ccg_out", [NCORES, G1, KN], BF16,
                             addr_space="Shared")
    ccv_in = nc.dram_tensor("ccv_in", [P, KN], BF16)
    ccv_out = nc.dram_tensor("ccv_out", [NCORES, P, KN], BF16,
                             addr_space="Shared")
    rbounce = nc.dram_tensor("rbounce", [2, KN], BF16)
    GROUPS = [list(range(NCORES))]

    tc_cm = tile.TileContext(nc)
    tc = tc_cm.__enter__()

    # ---- engine warmups: first use of an ACT table stalls; issue tiny
    # activations up front so table loads overlap the initial DMAs.
    warm, f_warm = tc.tile([1, 16], F32, name="warm")
    nc.vector.memset(warm[:], 1.0)
    for wf_i, wfunc in enumerate((AF.Identity, AF.Square, AF.Relu, AF.Exp,
                                  AF.Sqrt, AF.Ln, AF.Sigmoid)):
        wo_t, f_wo_t = tc.tile([1, 16], F32, name=f"warmo{wf_i}")
        nc.scalar.activation(wo_t[:], warm[:], wfunc)
        f_wo_t()
    f_warm()

    # ---- persistents (die at the very end), bottom of pool stack
    epsb, _f0 = tc.tile([1, 1], F32, name="epsb")
    nc.vector.memset(epsb[:], EPS * EPS)
    epsb_p, _f0b = tc.tile([P, 1], F32, name="epsb_p")
    nc.vector.memset(epsb_p[:], EPS * EPS)
    ones_col, _f1 = tc.tile([P, 1], BF16, name="ones_col")
    nc.any.memset(ones_col[:], 1.0)
    ones_row, _f2 = tc.tile([1, P], F32, name="ones_row")
    nc.any.memset(ones_row[:], 1.0)
    bkv_sb, _f3 = tc.tile([P, NT_KV], F32, name="bkv_sb")
    nc.sync.dma_start(bkv_sb[:], bkv[:])
    bw_sb, _f4 = tc.tile([1, 1], F32, name="bw_sb")
    nc.sync.dma_start(bw_sb[:], bws[:])
    bout_sb, _f5 = tc.tile([K, 1], F32, name="bout_sb")
    nc.sync.dma_start(bout_sb[:], boutt[:])
    wo_sb, _f6 = tc.tile([P, NT_KV * K], BF16, name="wo_sb")
    nc.sync.dma_start(wo_sb[:], wout[:])
    id20_sb, _f7 = tc.tile([NT_KN, NT_KN], BF16, name="id20_sb")
    nc.scalar.dma_start(id20_sb[:], ident[0:NT_KN, 0:NT_KN])

    # dies OUT-end
    vr_all, f_vr = tc.tile([P, NT_C, BL], BF16, name="vr_all")
    fr_all, f_fr = tc.tile([P, NT_C, BL], BF16, name="fr_all")
    # dies FE-end
    wf_all, f_wf = tc.tile([P, NT_KN, BL], BF16, name="wf_all")
    # die SIM-end (written after collectives)
    lhs2, f_lhs2 = tc.tile([P, NT_KN, 2], BF16, name="lhs2")
    inv_col, f_inv = tc.tile([P, NT_KN], F32, name="inv_col")
    kn_all, f_kn = tc.tile([P, NT_C, BL], BF16, name="kn_all")
    # die KV-end
    kT_all, f_kT = tc.tile([P, NT_C, BL], BF16, name="kT_all")
    sqk_all, f_sqk = tc.tile([P, NT_C, BL], BF16, name="sqk_all")
    xp_all, f_xp = tc.tile([P, NT_I, BL], BF16, name="xp_all")
    nc.gpsimd.dma_start(xp_all[:], xTt[:])
    # wkv blocks loaded post-loop so the statf stream goes first
    wblk0, f_wblk0 = tc.tile([P, NT_I, 512], BF16, name="wblk0")
    wblk1, f_wblk1 = tc.tile([P, NT_I, 512], BF16, name="wblk1")

    # ============ Phase P: Ek / Ev^T slabs (this core's 128 c's) =====
    H = NT_I // 2
    wek_sb, f_wek = tc.tile([P, NT_I, P], BF16, name="wek_sb")
    nc.sync.dma_start(wek_sb[:, 0:4, :], wekt[:, 0:4, :])
    wev_sb, f_wev = tc.tile([P, NT_I, P], BF16, name="wev_sb")
    nc.scalar.dma_start(wev_sb[:, 0:4, :], wevt[:, 0:4, :])
    bekc_sb, f_bek = tc.tile([P, 1], F32, name="bekc_sb")
    nc.scalar.dma_start(bekc_sb[:], bekc[:])
    bevc_sb, f_bev = tc.tile([P, 1], F32, name="bevc_sb")
    nc.scalar.dma_start(bevc_sb[:], bevc[:])
    wwc_sb, f_ww = tc.tile([P, 1], BF16, name="wwc_sb")
    nc.scalar.dma_start(wwc_sb[:], wwc[:])
    id_sb, f_id = tc.tile([P, P], BF16, name="id_sb")
    nc.scalar.dma_start(id_sb[:], ident[:])
    ek_slab, f_eks = tc.tile([P, KN], BF16, name="ek_slab")
    evt_slab, f_evs = tc.tile([P, KN], BF16, name="evt_slab")

    with tc.tile_pool(name="stp", bufs=3) as stp, \
         tc.tile_pool(name="pw", bufs=3) as pw, \
         tc.tile_pool(name="rowp", bufs=2) as rowp, \
         tc.tile_pool(name="ppk", bufs=2, space="PSUM") as ppk, \
         tc.tile_pool(name="prow", bufs=1, space="PSUM") as prow, \
         tc.tile_pool(name="ptp", bufs=2, space="PSUM") as ptp:
        for ch in range(NCH):
            cs = slice(ch * 512, (ch + 1) * 512)
            stch = stp.tile([P, NT_I, 512], BF16, tag="st")
            if ch == 0:
                # quarters so the first matmuls start sooner
                nc.sync.dma_start(stch[:, 0:4, :], statt[:, 0:4, cs])
                nc.scalar.dma_start(stch[:, 8:12, :], statt[:, 8:12, cs])
                nc.sync.dma_start(wek_sb[:, 4:16, :], wekt[:, 4:16, :])
                nc.scalar.dma_start(wev_sb[:, 4:16, :], wevt[:, 4:16, :])
                nc.sync.dma_start(stch[:, 4:8, :], statt[:, 4:8, cs])
                nc.scalar.dma_start(stch[:, 12:16, :], statt[:, 12:16, cs])
            else:
                nc.sync.dma_start(stch[:, 0:H, :], statt[:, 0:H, cs])
                nc.scalar.dma_start(stch[:, H:, :], statt[:, H:, cs])
            # Ek chunk
            ek_ps = ppk.tile([P, 512], F32, tag="ek")
            for i in range(NT_I):
                nc.tensor.matmul(ek_ps[:], wek_sb[:, i, :], stch[:, i, :],
                                 start=(i == 0), stop=(i == NT_I - 1))
            nc.scalar.activation(ek_slab[:, cs], ek_ps[:], AF.Identity,
                                 bias=bekc_sb[:])
            sqt = pw.tile([P, 512], BF16, tag="sq")
            nc.scalar.activation(sqt[:], ek_ps[:], AF.Square,
                                 bias=bekc_sb[:])
            sq_ps = prow.tile([1, 512], F32, tag="row")
            nc.tensor.matmul(sq_ps[:], ones_col[:], sqt[:])
            sq_row = rowp.tile([1, 512], BF16, tag="sqr")
            nc.vector.tensor_copy(sq_row[:], sq_ps[:])
            nc.scalar.dma_start(ccg_in[P:P + 1, cs], sq_row[:])
            # Ev chunk
            ev_ps = ppk.tile([P, 512], F32, tag="ev")
            for i in range(NT_I):
                nc.tensor.matmul(ev_ps[:], wev_sb[:, i, :], stch[:, i, :],
                                 start=(i == 0), stop=(i == NT_I - 1))
            evbf = pw.tile([P, 512], BF16, tag="ev")
            nc.scalar.activation(evbf[:], ev_ps[:], AF.Identity,
                                 bias=bevc_sb[:])
            wev_ps = prow.tile([1, 512], F32, tag="row")
            nc.tensor.matmul(wev_ps[:], wwc_sb[:], evbf[:])
            wev_row = rowp.tile([1, 512], BF16, tag="wvr")
            nc.vector.tensor_copy(wev_row[:], wev_ps[:])
            nc.scalar.dma_start(ccg_in[P + 1:P + 2, cs], wev_row[:])
            if ch == NCH - 1:
                # all of gather-1's input is now written (the ek slab
                # dump below only needs the ek activations, long done);
                # fire it before the remaining Ev^T work.
                nc.sync.dma_start(ccg_in[0:P, :], ek_slab[:])
                nc.gpsimd.collective_compute(
                    "AllGather", mybir.AluOpType.bypass,
                    replica_groups=GROUPS,
                    ins=[ccg_in[:].opt()], outs=[ccg_out[:].opt()])
            # Ev^T chunk (4 PE transposes via identity)
            tp_ps = ptp.tile([P, 512], BF16, tag="tp")
            for q in range(4):
                nc.tensor.transpose(tp_ps[:, q * P:(q + 1) * P],
                                    evbf[:, q * P:(q + 1) * P], id_sb[:])
            nc.scalar.copy(evt_slab[:, cs], tp_ps[:])
        # collective inputs first on each queue, then the KV streams
        nc.scalar.dma_start(ccv_in[:], evt_slab[:])
        nc.sync.dma_start(wblk0[:], wkvt[:, :, 0:512])
        nc.scalar.dma_start(wblk1[:], wkvt[:, :, 512:1024])
    f_evs()
    f_eks()
    f_id()
    f_ww()
    f_bev()
    f_bek()
    f_wev()
    f_wek()

    # KV weight blocks 2/3: issue the loads BEFORE the collectives —
    # HW-DGE triggers queued after a collective serialize behind it.
    kvw_cm = tc.tile_pool(name="wkvp", bufs=2)
    wkvp = kvw_cm.__enter__()
    wblk23 = []
    for mg in (2, 3):
        ms = slice(mg * 512, (mg + 1) * 512)
        wb = wkvp.tile([P, NT_I, 512], BF16, tag="wb", name=f"wb{mg}")
        nc.sync.dma_start(wb[:, 0:NT_I // 2, :], wkvt[:, 0:NT_I // 2, ms])
        nc.scalar.dma_start(wb[:, NT_I // 2:, :], wkvt[:, NT_I // 2:, ms])
        wblk23.append(wb)

    # ============ Collective 2 (overlaps KV; needed only at FE) =======
    nc.gpsimd.collective_compute(
        "AllGather", mybir.AluOpType.bypass, replica_groups=GROUPS,
        ins=[ccv_in[:].opt()], outs=[ccv_out[:].opt()])

    # ============ Phase KV: normalized kT, relu(vT) ==================
    # q-outer so each PSUM bank accumulates its 16 steps back-to-back;
    # the k sum-of-squares reduction is folded into the loop.
    with tc.tile_pool(name="pkv", bufs=3, space="PSUM") as pkv, \
         tc.tile_pool(name="pssk", bufs=1, space="PSUM") as pssk, \
         tc.tile_pool(name="kvw", bufs=2) as kvw, \
         tc.tile_pool(name="pbc", bufs=1, space="PSUM") as pbc:
        ssk = pssk.tile([1, BL], F32)
        for mg in range(4):
            wblk = (wblk0, wblk1, wblk23[0], wblk23[1])[mg]
            for q in range(4):
                m = mg * 4 + q
                kv_ps = pkv.tile([P, BL], F32, tag="kv", name=f"kv{m}")
                for i in range(NT_I):
                    nc.tensor.matmul(
                        kv_ps[:], wblk[:, i, q * P:(q + 1) * P],
                        xp_all[:, i, :],
                        start=(i == 0), stop=(i == NT_I - 1))
                if m < NT_C:
                    nc.scalar.activation(
                        kT_all[:, m, :], kv_ps[:], AF.Identity,
                        bias=bkv_sb[:, m:m + 1])
                    nc.scalar.activation(
                        sqk_all[:, m, :], kv_ps[:], AF.Square,
                        bias=bkv_sb[:, m:m + 1])
                    nc.tensor.matmul(ssk[:], ones_col[:],
                                     sqk_all[:, m, :],
                                     start=(m == 0), stop=(m == NT_C - 1))
                else:
                    nc.scalar.activation(
                        vr_all[:, m - NT_C, :], kv_ps[:], AF.Relu,
                        bias=bkv_sb[:, m:m + 1])
            if mg == 1:
                # everything for k-normalization is ready at mg1-end;
                # run the chain under the mg2/3 matmuls.
                nk = kvw.tile([1, BL], F32, tag="nk")
                nc.scalar.activation(nk[:], ssk[:], AF.Sqrt, bias=epsb[:])
                invk = kvw.tile([1, BL], F32, tag="invk")
                nc.vector.reciprocal(invk[:], nk[:])
                bc = pbc.tile([P, BL], F32)
                nc.tensor.matmul(bc[:], ones_row[:], invk[:])
                for m in range(NT_C):
                    nc.vector.tensor_mul(kn_all[:, m, :],
                                         kT_all[:, m, :], bc[:])
    kvw_cm.__exit__(None, None, None)
    # inv_col = 1/sqrt(sum over ranks of sq partials + eps^2); wEv gate
    # row -> lhs2[:, :, 1]. The rank partials (rows P/P+1 of each
    # gathered block) are summed straight into [p, j] layout with
    # software-DGE accumulate-DMAs on the otherwise idle gpsimd queue.
    with tc.tile_pool(name="colw", bufs=1) as colw:
        sq_acc = colw.tile([P, NT_KN], F32, tag="sqa")
        nc.vector.memset(sq_acc[:], 0.0)
        wv_acc = colw.tile([P, NT_KN], F32, tag="wva")
        nc.vector.memset(wv_acc[:], 0.0)
        for m in range(NCORES):
            nc.gpsimd.dma_start(
                sq_acc[:],
                ccg_out[m, P, :].rearrange("(j p) -> p j", p=P),
                accum_op=mybir.AluOpType.add)
            nc.gpsimd.dma_start(
                wv_acc[:],
                ccg_out[m, P + 1, :].rearrange("(j p) -> p j", p=P),
                accum_op=mybir.AluOpType.add)
        nrm = colw.tile([P, NT_KN], F32, tag="nrm")
        nc.scalar.activation(nrm[:], sq_acc[:], AF.Sqrt, bias=epsb_p[:])
        nc.vector.reciprocal(inv_col[:], nrm[:])
        nc.any.memset(lhs2[:], 1.0)
        nc.vector.tensor_copy(lhs2[:, :, 1], wv_acc[:])

    f_wblk1()
    f_wblk0()
    f_xp()
    f_sqk()
    f_kT()

    # ============ Fused SIM + GATE + WF ==============================
    with tc.tile_pool(name="ekp", bufs=3) as ekp, \
         tc.tile_pool(name="gw", bufs=2) as gw, \
         tc.tile_pool(name="esw", bufs=8) as esw, \
         tc.tile_pool(name="psim", bufs=3, space="PSUM") as psim, \
         tc.tile_pool(name="pg", bufs=2, space="PSUM") as pg, \
         tc.tile_pool(name="pbc2", bufs=1, space="PSUM") as pbc2:
        for k in range(K):
            ks = slice(k * 512, (k + 1) * 512)
            ekt = ekp.tile([P, NT_C, 512], BF16, tag="ek")
            for m in range(NT_C):
                eng = nc.sync if (k * NT_C + m) % 2 else nc.scalar
                eng.dma_start(ekt[:, m, :], ccg_out[m, 0:P, ks])
            gse = pg.tile([1, BL], F32, tag="gse")
            gtg = pg.tile([1, BL], F32, tag="gtg")
            es_list = []
            for j in range(TPK):
                kt = k * TPK + j
                ps = psim.tile([P, BL], F32, tag="simps")
                for m in range(NT_C):
                    nc.tensor.matmul(
                        ps[:], ekt[:, m, j * P:(j + 1) * P],
                        kn_all[:, m, :],
                        start=(m == 0), stop=(m == NT_C - 1))
                es = esw.tile([P, BL], BF16, tag="esw")
                nc.scalar.activation(es[:], ps[:], AF.Exp,
                                     scale=inv_col[:, kt:kt + 1])
                es_list.append(es)
                nc.tensor.matmul(gse[:], lhs2[:, kt, 0:1], es[:],
                                 start=(j == 0), stop=(j == TPK - 1))
                nc.tensor.matmul(gtg[:], lhs2[:, kt, 1:2], es[:],
                                 start=(j == 0), stop=(j == TPK - 1))
            lgse = gw.tile([1, BL], F32, tag="lgse")
            nc.scalar.activation(lgse[:], gse[:], AF.Ln)
            rs = gw.tile([1, BL], F32, tag="rs")
            nc.scalar.activation(rs[:], lgse[:], AF.Exp, scale=-1.0)
            tg = gw.tile([1, BL], F32, tag="tg")
            nc.vector.tensor_mul(tg[:], gtg[:], rs[:])
            fwk = gw.tile([1, BL], F32, tag="fwk")
            nc.scalar.activation(fwk[:], tg[:], AF.Sigmoid,
                                 bias=bw_sb[0:1, 0:1])
            sk = gw.tile([1, BL], F32, tag="sk")
            nc.vector.tensor_mul(sk[:], fwk[:], rs[:])
            bcs = pbc2.tile([P, BL], F32, tag="bcs")
            nc.tensor.matmul(bcs[:], ones_row[:], sk[:])
            bcs_sb = gw.tile([P, BL], BF16, tag="bcssb")
            nc.scalar.copy(bcs_sb[:], bcs[:])
            for j in range(TPK):
                kt = k * TPK + j
                nc.vector.tensor_mul(wf_all[:, kt, :], es_list[j],
                                     bcs_sb[:])
    f_kn()
    f_inv()
    f_lhs2()

    # ============ Phase FE + OUT =====================================
    # the final projection's v-half accumulates up front; each fE tile
    # feeds its po term right after its relu.
    with tc.tile_pool(name="evp", bufs=2) as evp, \
         tc.tile_pool(name="ow", bufs=1) as ow, \
         tc.tile_pool(name="pfe", bufs=3, space="PSUM") as pfe, \
         tc.tile_pool(name="pout", bufs=1, space="PSUM") as pout:
        po = pout.tile([K, BL], F32)
        for j in range(NT_C):
            nc.tensor.matmul(po[:], wo_sb[:, j * K:(j + 1) * K],
                             vr_all[:, j, :],
                             start=(j == 0), stop=False)
        for mc in range(NT_C):
            evtt = evp.tile([P, KN], BF16, tag="evt")
            eng = nc.sync if mc % 2 else nc.scalar
            eng.dma_start(evtt[:], ccv_out[mc])
            ps = pfe.tile([P, BL], F32, tag="feps")
            for kt in range(NT_KN):
                nc.tensor.matmul(
                    ps[:], evtt[:, kt * P:(kt + 1) * P],
                    wf_all[:, kt, :],
                    start=(kt == 0), stop=(kt == NT_KN - 1))
            nc.scalar.activation(fr_all[:, mc, :], ps[:], AF.Relu)
            nc.tensor.matmul(po[:], wo_sb[:, (NT_C + mc) * K:
                                          (NT_C + mc + 1) * K],
                             fr_all[:, mc, :],
                             start=False, stop=(mc == NT_C - 1))
        osb = ow.tile([K, BL], F32)
        nc.scalar.activation(osb[:], po[:], AF.Identity,
                             bias=bout_sb[:])
        nc.sync.dma_start(outT[:], osb[:])
    f_wf()
    f_fr()
    f_vr()
    _f7()
    _f6()
    _f5()
    _f4()
    _f3()
    _f2()
    _f1()
    _f0b()
    _f0()

    tc_cm.__exit__(None, None, None)
    nc.compile()
    return nc


def _tile_rows(a):
    """[NT_I*P, n] -> [P, NT_I, n]: row (i*P + p) -> [p, i]."""
    n = a.shape[1]
    return np.ascontiguousarray(
        a.reshape(NT_I, P, n).transpose(1, 0, 2))


def _host_prep(inputs):
    bf = ml_dtypes.bfloat16
    x_last = np.asarray(inputs["x"])[:, -1, :]  # [B, CH] f32
    wekT = np.asarray(inputs["WEk"]).T  # [CH, C]
    wevT = np.asarray(inputs["WEv"]).T
    shared = {
        "wkvt": _tile_rows(
            np.concatenate([inputs["Wk"], inputs["Wv"]], axis=0).T
        ).astype(bf),
        "statt": _tile_rows(
            np.asarray(inputs["static"]).transpose(1, 0, 2).reshape(CH, KN)
        ).astype(bf),
        "bkv": np.ascontiguousarray(
            np.concatenate([inputs["bk"], inputs["bv"]]).reshape(NT_KV, P).T),
        "ident": np.eye(P, dtype=bf),
        "wout": np.ascontiguousarray(
            np.asarray(inputs["Wout"]).T.reshape(NT_KV, P, K)
            .transpose(1, 0, 2).reshape(P, NT_KV * K)).astype(bf),
        "bws": np.asarray(inputs["bw"], dtype=np.float32).reshape(1, 1),
        "boutt": np.asarray(inputs["bout"], dtype=np.float32).reshape(K, 1),
    }
    in_maps = []
    for r in range(NCORES):
        cslc = slice(r * P, (r + 1) * P)
        m = dict(shared)
        m["xTt"] = _tile_rows(
            np.ascontiguousarray(x_last[r * BL:(r + 1) * BL].T)).astype(bf)
        m["wekt"] = _tile_rows(
            np.ascontiguousarray(wekT[:, cslc])).astype(bf)
        m["wevt"] = _tile_rows(
            np.ascontiguousarray(wevT[:, cslc])).astype(bf)
        m["bekc"] = np.ascontiguousarray(
            np.asarray(inputs["bEk"], dtype=np.float32)[cslc].reshape(P, 1))
        m["bevc"] = np.ascontiguousarray(
            np.asarray(inputs["bEv"], dtype=np.float32)[cslc].reshape(P, 1))
        m["wwc"] = np.ascontiguousarray(
            np.asarray(inputs["Ww"])[0, cslc].reshape(P, 1)).astype(bf)
        in_maps.append(m)
    return in_maps


def kernel(**inputs):
    if "nc" not in _CACHE:
        _CACHE["nc"] = _build_nc()
    nc = _CACHE["nc"]
    in_maps = _host_prep(inputs)
    res = bass_utils.run_bass_kernel_spmd(
        nc, in_maps, core_ids=list(range(NCORES)), trace=False)
    out = np.concatenate(
        [res.results[r]["outT"].T for r in range(NCORES)], axis=0)
    return np.ascontiguousarray(out[:, :, None], dtype=np.float32)
